# revision 3
# baseline (speedup 1.0000x reference)
"""CASS block (LayerNorm + gradient-selected scan + fc1/dwconv/gelu/fc2 + residual)
on 8 TRN2 NeuronCores, pure data parallel over the batch.

v2 — PE-centric restructure.  Key algebra: with per-pixel LN stats (mu, rstd),
    u = LN(x) @ W1 + b1 = (rstd .* [x; mu]) @ [W1g; -s1g] + b1aug
so the per-pixel rstd is folded into the matmul *rhs* (one DVE scale of the
channel-major input columns) instead of the psum output (3x larger).  The
3-tap depthwise conv that follows fc1 is linear, so it is folded into the
fc1 matmul itself: conv(u)[l] = sum_k (W1g .* w_k)^T xr[l+k-1], i.e. three
shifted accumulating matmuls per output block against one zero-guarded
channel-major input (guard columns give the conv's zero padding for free).
Gelu (with the fc1/dwconv biases deferred into its bias operand, plus 2
boundary-column fixups) reads the accumulated PSUM directly.  fc2 runs
channel-major with w2 as the stationary operand, and the kernel emits only
the block output F = fc2(gelu(...)); the residual y = x + F + fc2_b is an
elementwise epilogue done host-side in fp32 (exact x, no back-transpose).

The gradient selector: for uniform gamma the "gray" image mean_c(LN(x)) is a
constant, so grad_h = grad_v = 0, the MLP logits tie, softmax gives exactly
0.25 each in fp32, and argmax -> idx 0 for every sample: the 'v' (transpose)
branch is dead.  The device kernel therefore always scans row-major; a host
fallback handles non-uniform gamma by pre-transposing flagged samples (the
row-major reshape of the result is orientation-identical, so y = x + F(x_dev)
recovers the reference output exactly).
"""

import numpy as np
import ml_dtypes

import concourse.mybir as mybir
import concourse.tile as tile
from concourse import bacc

B, H, W, C = 32, 56, 56, 192
D = 384                      # D_INNER
NCORES = 8
S = B // NCORES              # samples per core
L = H * W                    # 3136 pixels per sample
PT = 128                     # pixels per partition tile (stats)
NT = (L + PT - 1) // PT      # 25 pixel tiles (24 full + 64 tail)
TAIL = L - (NT - 1) * PT     # 64
NBLK = 7                     # fc1/fc2 N blocks per sample
NB = L // NBLK               # 448 columns per matmul
EPS = 1e-5
F32 = mybir.dt.float32
BF16 = mybir.dt.bfloat16
AL = mybir.AluOpType
AF = mybir.ActivationFunctionType

_CACHE = {}


def _build_nc():
    nc = bacc.Bacc()
    xs_d = nc.declare_dram_parameter("xs", [S * L, C], BF16, isOutput=False)
    xt_d = nc.declare_dram_parameter("xt", [S, 128, 2, L + 2], BF16,
                                     isOutput=False)
    w1a_d = nc.declare_dram_parameter("w1a", [128, 3, D], BF16, isOutput=False)
    w1b_d = nc.declare_dram_parameter("w1b", [65, 3, D], BF16, isOutput=False)
    w2_d = nc.declare_dram_parameter("w2", [128, 3, C], BF16, isOutput=False)
    gb3_d = nc.declare_dram_parameter("gb3", [128, 3, 3], F32, isOutput=False)
    f_d = nc.declare_dram_parameter("F", [S, 128, 2, L], BF16, isOutput=True)

    with tile.TileContext(nc) as tc, \
         tc.tile_pool(name="const", bufs=1) as const, \
         tc.tile_pool(name="xs", bufs=2) as xspool, \
         tc.tile_pool(name="xt", bufs=2) as xtpool, \
         tc.tile_pool(name="stat", bufs=2) as stat, \
         tc.tile_pool(name="rb", bufs=2) as rbpool, \
         tc.tile_pool(name="t", bufs=2) as tpool, \
         tc.tile_pool(name="f", bufs=2) as fpool, \
         tc.tile_pool(name="pf1", bufs=4, space="PSUM") as pf1, \
         tc.tile_pool(name="pf2", bufs=4, space="PSUM") as pf2:

        w1a = const.tile([128, 3, D], BF16)
        w1b = const.tile([65, 3, D], BF16)
        w2 = const.tile([128, 3, C], BF16)
        gb3 = const.tile([128, 3, 3], F32)
        eps_sb = const.tile([128, 1], F32)
        nc.sync.dma_start(out=w1a, in_=w1a_d[:, :, :])
        nc.sync.dma_start(out=w1b, in_=w1b_d[:, :, :])
        nc.sync.dma_start(out=w2, in_=w2_d[:, :, :])
        nc.sync.dma_start(out=gb3, in_=gb3_d[:, :, :])
        nc.vector.memset(eps_sb, EPS)

        state = {}

        def pre_sample(s):
            base = s * L

            # ---- pixel-major bf16 x (stats only) + LN stats
            #      pack[:,0,k]=mu_k, pack[:,1,k]=var_k->rstd_k
            bns = stat.tile([128, NT, 6], F32)
            pack = stat.tile([128, 2, 64], F32)
            packb = stat.tile([128, 2, 64], BF16)
            nc.vector.memset(pack, 0.0)
            xs_sb = xspool.tile([128, NT, C], BF16)
            for j in range(12):
                nc.sync.dma_start(
                    out=xs_sb[:, 2 * j:2 * j + 2, :],
                    in_=xs_d[base + j * 256: base + (j + 1) * 256, :]
                        .rearrange("(two p) c -> p two c", p=128),
                )
            nc.sync.dma_start(
                out=xs_sb[0:TAIL, NT - 1, :],
                in_=xs_d[base + (NT - 1) * PT: base + L, :],
            )
            xt = xtpool.tile([128, 2, L + 2], BF16)
            nc.sync.dma_start(out=xt, in_=xt_d[s, :, :, :])
            for k in range(NT - 1):
                nc.vector.bn_stats(out=bns[:, k:k + 1, :],
                                   in_=xs_sb[:, k:k + 1, :])
                nc.vector.bn_aggr(out=pack[:, :, k], in_=bns[:, k:k + 1, :])
            nc.vector.bn_stats(out=bns[0:TAIL, NT - 1:NT, :],
                               in_=xs_sb[0:TAIL, NT - 1:NT, :])
            nc.vector.bn_aggr(out=pack[0:TAIL, :, NT - 1],
                              in_=bns[0:TAIL, NT - 1:NT, :])
            # rstd = 1/sqrt(var+eps) in place
            nc.scalar.activation(out=pack[:, 1, 0:NT], in_=pack[:, 1, 0:NT],
                                 func=AF.Sqrt, bias=eps_sb[:, :], scale=1.0)
            nc.vector.reciprocal(out=pack[:, 1, 0:NT], in_=pack[:, 1, 0:NT])
            nc.vector.tensor_copy(out=packb, in_=pack)

            # ---- one small xbar transpose + row-linearize DMAs:
            #      mu row -> xt[64,1,:] (fc1 aug row), rstd row -> broadcast
            packT = stat.tile([128, 128], BF16)
            nc.sync.dma_start(out=packT,
                              in_=packb.rearrange("p a b -> p (a b)"),
                              transpose=True)
            nc.sync.dma_start(out=xt[64:65, 1, 1:1 + (NT - 1) * PT],
                              in_=packT[0:NT - 1, :])
            nc.sync.dma_start(out=xt[64:65, 1, 1 + (NT - 1) * PT:1 + L],
                              in_=packT[NT - 1:NT, 0:TAIL])
            rrow = stat.tile([1, L], BF16)
            nc.sync.dma_start(out=rrow[0:1, 0:(NT - 1) * PT],
                              in_=packT[64:64 + NT - 1, :])
            nc.sync.dma_start(out=rrow[0:1, (NT - 1) * PT:L],
                              in_=packT[64 + NT - 1:64 + NT, 0:TAIL])
            rstd_b = rbpool.tile([128, L], BF16)
            nc.gpsimd.partition_broadcast(rstd_b, rrow[0:1, :])

            # ---- fold rstd into the fc1 rhs: one column scale of both
            #      channel planes (guard cols 0 and L+1 stay zero)
            nc.vector.tensor_tensor(out=xt[:, 0, 1:1 + L], in0=xt[:, 0, 1:1 + L],
                                    in1=rstd_b, op=AL.mult)
            nc.vector.tensor_tensor(out=xt[:, 1, 1:1 + L], in0=xt[:, 1, 1:1 + L],
                                    in1=rstd_b, op=AL.mult)
            state[s] = xt

        def main_sample(s):
            xt = state.pop(s)

            # ---- fc1 + depthwise conv fused on PE: 3 shifted tap matmuls
            #      (x2 K-chunks) accumulate conv(u) per block; gelu (with
            #      deferred biases) evacuates PSUM directly.
            t = tpool.tile([128, 3, L], BF16)
            for blk in range(NBLK):
                cs = blk * NB
                for m in range(3):
                    pt_ = pf1.tile([128, NB], F32)
                    for k in range(3):
                        nc.tensor.matmul(pt_,
                                         lhsT=w1a[:, k, m * 128:(m + 1) * 128],
                                         rhs=xt[:, 0, cs + k:cs + k + NB],
                                         start=(k == 0), stop=False)
                        nc.tensor.matmul(pt_,
                                         lhsT=w1b[:, k, m * 128:(m + 1) * 128],
                                         rhs=xt[0:65, 1, cs + k:cs + k + NB],
                                         start=False, stop=(k == 2))
                    nc.scalar.activation(out=t[:, m, cs:cs + NB], in_=pt_,
                                         func=AF.Gelu, bias=gb3[:, m, 0:1],
                                         scale=1.0)
                    if blk == 0:
                        nc.scalar.activation(out=t[:, m, 0:1], in_=pt_[:, 0:1],
                                             func=AF.Gelu, bias=gb3[:, m, 1:2],
                                             scale=1.0)
                    if blk == NBLK - 1:
                        nc.scalar.activation(out=t[:, m, L - 1:L],
                                             in_=pt_[:, NB - 1:NB],
                                             func=AF.Gelu, bias=gb3[:, m, 2:3],
                                             scale=1.0)

            # ---- fc2 channel-major: w2 chunks stationary, t moving; emits
            #      F = t @ W2 (bias + residual live on the host)
            f_sb = fpool.tile([128, 2, L], BF16)
            for blk in range(NBLK):
                cs = blk * NB
                p0 = pf2.tile([128, NB], F32, tag="p0")
                p1 = pf2.tile([64, NB], F32, tag="p1")
                for d3 in range(3):
                    nc.tensor.matmul(p0, lhsT=w2[:, d3, 0:128],
                                     rhs=t[:, d3, cs:cs + NB],
                                     start=(d3 == 0), stop=(d3 == 2))
                for d3 in range(3):
                    nc.tensor.matmul(p1, lhsT=w2[:, d3, 128:192],
                                     rhs=t[:, d3, cs:cs + NB],
                                     start=(d3 == 0), stop=(d3 == 2))
                nc.scalar.copy(out=f_sb[:, 0, cs:cs + NB], in_=p0)
                nc.scalar.copy(out=f_sb[0:64, 1, cs:cs + NB], in_=p1)
            nc.sync.dma_start(out=f_d[s, :, :, :], in_=f_sb)

        # stats/rstd prep of sample s+1 is emitted ahead of sample s's heavy
        # phases so the stats chain never gates the next sample's fc1
        pre_sample(0)
        for s in range(S):
            if s + 1 < S:
                pre_sample(s + 1)
            main_sample(s)
    nc.finalize()
    return nc


def _get_nc():
    if "nc" not in _CACHE:
        _CACHE["nc"] = _build_nc()
    return _CACHE["nc"]


def _host_params(gamma, beta, fc1_w, fc1_b, dw_w, dw_b, fc2_w, fc2_b):
    bf = ml_dtypes.bfloat16
    w1g = (fc1_w * gamma[:, None]).astype(np.float32)          # [192, 384]
    s1g = w1g.sum(0)                                           # [384]
    b1aug = (beta @ fc1_w + fc1_b).astype(np.float32)          # [384]
    dwtaps = dw_w[:, 0, :].T.astype(np.float32)                # [3, 384]
    w1a = (w1g[0:128][:, None, :] * dwtaps[None, :, :]).astype(bf)
    w1b_base = np.concatenate([w1g[128:192], -s1g[None, :]], 0)  # [65, 384]
    w1b = (w1b_base[:, None, :] * dwtaps[None, :, :]).astype(bf)
    w2 = np.ascontiguousarray(
        fc2_w.reshape(3, 128, C).transpose(1, 0, 2)).astype(bf)  # [128,3,192]
    w0, w1_, w2_ = dwtaps[0], dwtaps[1], dwtaps[2]
    gb_int = dw_b + b1aug * (w0 + w1_ + w2_)
    gb_l = dw_b + b1aug * (w1_ + w2_)        # col 0: tap0 falls on zero pad
    gb_r = dw_b + b1aug * (w0 + w1_)         # col L-1: tap2 falls on zero pad
    gb3 = np.ascontiguousarray(
        np.stack([gb_int, gb_l, gb_r], -1).reshape(3, 128, 3)
        .transpose(1, 0, 2)).astype(np.float32)                # [128, 3, 3]
    return dict(w1a=np.ascontiguousarray(w1a),
                w1b=np.ascontiguousarray(w1b), w2=w2, gb3=gb3)


def _host_xt(x_dev):
    """Channel-major bf16 copy of x: [B, 128, 2, L+2] with zero guard columns
    at 0 and L+1 (the conv's zero padding); [:, 64:, 1, :] holds the runtime
    mu row (slot 64) and padding, zero-filled here."""
    bf = ml_dtypes.bfloat16
    nb = x_dev.shape[0]
    arr = np.ascontiguousarray(
        x_dev.reshape(nb, L, C).transpose(0, 2, 1)).astype(bf)  # [nb, 192, L]
    xt = np.zeros((nb, 128, 2, L + 2), dtype=bf)
    xt[:, :, 0, 1:1 + L] = arr[:, 0:128]
    xt[:, 0:64, 1, 1:1 + L] = arr[:, 128:192]
    return xt


def _selector_flags(x, gamma, beta, sel_w1, sel_b1, sel_w2, sel_b2):
    """Exact numpy replica of the reference direction selector. Only used
    when gamma is non-uniform (otherwise the scores tie and idx==0 always)."""
    xf = x.astype(np.float32)
    mu = xf.mean(-1, keepdims=True)
    var = ((xf - mu) ** 2).mean(-1, keepdims=True)
    xn = (xf - mu) / np.sqrt(var + EPS) * gamma + beta
    xg = xn.mean(-1)
    gh = np.abs(xg[:, :, 1:] - xg[:, :, :-1]).mean(axis=(1, 2))
    gv = np.abs(xg[:, 1:, :] - xg[:, :-1, :]).mean(axis=(1, 2))
    scores = np.stack([gh, gv, 0.8 * (gh + gv) * 0.5, np.abs(gh - gv)], 1)
    hdn = np.maximum(scores @ sel_w1 + sel_b1, 0.0)
    logits = hdn @ sel_w2 + sel_b2
    ex = np.exp(logits - logits.max(1, keepdims=True))
    probs = ex / ex.sum(1, keepdims=True)
    return probs.argmax(1) % 4 == 1


def build_in_maps(inputs):
    """Shared by kernel() and test harnesses: host preprocessing + sharding.
    Returns (in_maps, x, x_dev, flags)."""
    bf = ml_dtypes.bfloat16
    x = np.asarray(inputs["x"], dtype=np.float32)
    gamma = np.asarray(inputs["gamma"], np.float32)
    beta = np.asarray(inputs["beta"], np.float32)
    params = _host_params(
        gamma, beta,
        np.asarray(inputs["fc1_w"], np.float32),
        np.asarray(inputs["fc1_b"], np.float32),
        np.asarray(inputs["dw_w"], np.float32),
        np.asarray(inputs["dw_b"], np.float32),
        np.asarray(inputs["fc2_w"], np.float32),
        np.asarray(inputs["fc2_b"], np.float32),
    )

    # Routing: uniform gamma => gray image is constant => scores tie => idx 0
    # for every sample (see module docstring).  Otherwise compute the selector
    # on host and pre-transpose flagged samples (mathematically exact fixup).
    if np.ptp(gamma) == 0.0:
        flags = np.zeros(B, dtype=bool)
    else:
        flags = _selector_flags(
            x, gamma, beta,
            np.asarray(inputs["sel_w1"], np.float32),
            np.asarray(inputs["sel_b1"], np.float32),
            np.asarray(inputs["sel_w2"], np.float32),
            np.asarray(inputs["sel_b2"], np.float32))
    x_dev = x
    if flags.any():
        x_dev = x.copy()
        x_dev[flags] = np.swapaxes(x_dev[flags], 1, 2)

    xt = _host_xt(x_dev)
    xs = x_dev.reshape(B, L, C).astype(bf)
    in_maps = []
    for i in range(NCORES):
        m = {"xs": np.ascontiguousarray(
                 xs[S * i:S * (i + 1)].reshape(S * L, C)),
             "xt": xt[S * i:S * (i + 1)]}
        m.update(params)
        in_maps.append(m)
    return in_maps, x, x_dev, flags


def kernel(**inputs):
    from concourse.bass_utils import run_bass_kernel_spmd

    in_maps, x, x_dev, flags = build_in_maps(inputs)
    nc = _get_nc()
    res = run_bass_kernel_spmd(nc, in_maps, list(range(NCORES)))
    fcm = np.empty((B, 192, L), np.float32)
    for i, r in enumerate(res.results):
        fcm[S * i:S * (i + 1), 0:128] = r["F"][:, :, 0, :]
        fcm[S * i:S * (i + 1), 128:192] = r["F"][:, 0:64, 1, :]
    out = fcm.transpose(0, 2, 1).reshape(B, H, W, C)
    fc2_b = np.asarray(inputs["fc2_b"], np.float32)
    # device computed F(x_dev); reference wants x + F(x_dev) + fc2_b
    # (row-major unscan orientation is identical for flagged samples)
    y = x + out + fc2_b
    return y.astype(np.float32)


# revision 4
# speedup vs baseline: 1.4095x; 1.4095x over previous
"""CASS block (LayerNorm + gradient-selected scan + fc1/dwconv/gelu/fc2 + residual)
on 8 TRN2 NeuronCores, pure data parallel over the batch.

v2 — PE-centric restructure.  Key algebra: with per-pixel LN stats (mu, rstd),
    u = LN(x) @ W1 + b1 = (rstd .* [x; mu]) @ [W1g; -s1g] + b1aug
so the per-pixel rstd is folded into the matmul *rhs* (one DVE scale of the
channel-major input columns) instead of the psum output (3x larger).  The
3-tap depthwise conv that follows fc1 is linear, so it is folded into the
fc1 matmul itself: conv(u)[l] = sum_k (W1g .* w_k)^T xr[l+k-1], i.e. three
shifted accumulating matmuls per output block against one zero-guarded
channel-major input (guard columns give the conv's zero padding for free).
Gelu (with the fc1/dwconv biases deferred into its bias operand, plus 2
boundary-column fixups) reads the accumulated PSUM directly.  fc2 runs
channel-major with w2 as the stationary operand, and the kernel emits only
the block output F = fc2(gelu(...)); the residual y = x + F + fc2_b is an
elementwise epilogue done host-side in fp32 (exact x, no back-transpose).

The gradient selector: for uniform gamma the "gray" image mean_c(LN(x)) is a
constant, so grad_h = grad_v = 0, the MLP logits tie, softmax gives exactly
0.25 each in fp32, and argmax -> idx 0 for every sample: the 'v' (transpose)
branch is dead.  The device kernel therefore always scans row-major; a host
fallback handles non-uniform gamma by pre-transposing flagged samples (the
row-major reshape of the result is orientation-identical, so y = x + F(x_dev)
recovers the reference output exactly).
"""

import numpy as np
import ml_dtypes

import concourse.mybir as mybir
import concourse.tile as tile
from concourse import bacc

B, H, W, C = 32, 56, 56, 192
D = 384                      # D_INNER
NCORES = 8
S = B // NCORES              # samples per core
L = H * W                    # 3136 pixels per sample
PT = 128                     # pixels per partition tile (stats)
NT = (L + PT - 1) // PT      # 25 pixel tiles (24 full + 64 tail)
TAIL = L - (NT - 1) * PT     # 64
NBLK = 7                     # fc1/fc2 N blocks per sample
NB = L // NBLK               # 448 columns per matmul
EPS = 1e-5
F32 = mybir.dt.float32
BF16 = mybir.dt.bfloat16
AL = mybir.AluOpType
AF = mybir.ActivationFunctionType

_CACHE = {}


def _build_nc():
    nc = bacc.Bacc()
    xs_d = nc.declare_dram_parameter("xs", [S * L, C], BF16, isOutput=False)
    xt_d = nc.declare_dram_parameter("xt", [S, 128, 2, L + 2], BF16,
                                     isOutput=False)
    w1a_d = nc.declare_dram_parameter("w1a", [128, 3, D], BF16, isOutput=False)
    w1b_d = nc.declare_dram_parameter("w1b", [65, 3, D], BF16, isOutput=False)
    w2_d = nc.declare_dram_parameter("w2", [128, 3, C], BF16, isOutput=False)
    gb3_d = nc.declare_dram_parameter("gb3", [128, 3, 3], F32, isOutput=False)
    f_d = nc.declare_dram_parameter("F", [S, 128, 2, L], BF16, isOutput=True)

    with tile.TileContext(nc) as tc, \
         tc.tile_pool(name="const", bufs=1) as const, \
         tc.tile_pool(name="xs", bufs=2) as xspool, \
         tc.tile_pool(name="xt", bufs=2) as xtpool, \
         tc.tile_pool(name="stat", bufs=2) as stat, \
         tc.tile_pool(name="rb", bufs=2) as rbpool, \
         tc.tile_pool(name="t", bufs=2) as tpool, \
         tc.tile_pool(name="f", bufs=2) as fpool, \
         tc.tile_pool(name="pf1", bufs=4, space="PSUM") as pf1, \
         tc.tile_pool(name="pf2", bufs=2, space="PSUM") as pf2:

        w1a = const.tile([128, 3, D], BF16)
        w1b = const.tile([65, 3, D], BF16)
        w2 = const.tile([128, 3, C], BF16)
        gb3 = const.tile([128, 3, 3], F32)
        eps_sb = const.tile([128, 1], F32)
        nc.sync.dma_start(out=w1a, in_=w1a_d[:, :, :])
        nc.sync.dma_start(out=w1b, in_=w1b_d[:, :, :])
        nc.sync.dma_start(out=w2, in_=w2_d[:, :, :])
        nc.sync.dma_start(out=gb3, in_=gb3_d[:, :, :])
        nc.vector.memset(eps_sb, EPS)

        state = {}

        def pre_sample(s):
            base = s * L

            # ---- pixel-major bf16 x (stats only) + LN stats
            #      pack[:,0,k]=mu_k, pack[:,1,k]=var_k->rstd_k
            bns = stat.tile([128, NT, 6], F32)
            pack = stat.tile([128, 2, 64], F32)
            packb = stat.tile([128, 2, 64], BF16)
            nc.vector.memset(pack, 0.0)
            xs_sb = xspool.tile([128, NT, C], BF16)
            for j in range(12):
                nc.sync.dma_start(
                    out=xs_sb[:, 2 * j:2 * j + 2, :],
                    in_=xs_d[base + j * 256: base + (j + 1) * 256, :]
                        .rearrange("(two p) c -> p two c", p=128),
                )
            nc.sync.dma_start(
                out=xs_sb[0:TAIL, NT - 1, :],
                in_=xs_d[base + (NT - 1) * PT: base + L, :],
            )
            xt = xtpool.tile([128, 2, L + 2], BF16)
            nc.sync.dma_start(out=xt, in_=xt_d[s, :, :, :])
            for k in range(NT - 1):
                nc.vector.bn_stats(out=bns[:, k:k + 1, :],
                                   in_=xs_sb[:, k:k + 1, :])
                nc.vector.bn_aggr(out=pack[:, :, k], in_=bns[:, k:k + 1, :])
            nc.vector.bn_stats(out=bns[0:TAIL, NT - 1:NT, :],
                               in_=xs_sb[0:TAIL, NT - 1:NT, :])
            nc.vector.bn_aggr(out=pack[0:TAIL, :, NT - 1],
                              in_=bns[0:TAIL, NT - 1:NT, :])
            # rstd = 1/sqrt(var+eps) in place
            nc.scalar.activation(out=pack[:, 1, 0:NT], in_=pack[:, 1, 0:NT],
                                 func=AF.Sqrt, bias=eps_sb[:, :], scale=1.0)
            nc.vector.reciprocal(out=pack[:, 1, 0:NT], in_=pack[:, 1, 0:NT])
            nc.vector.tensor_copy(out=packb, in_=pack)

            # ---- one small xbar transpose + row-linearize DMAs:
            #      mu row -> xt[64,1,:] (fc1 aug row), rstd row -> broadcast
            packT = stat.tile([128, 128], BF16)
            nc.sync.dma_start(out=packT,
                              in_=packb.rearrange("p a b -> p (a b)"),
                              transpose=True)
            nc.sync.dma_start(out=xt[64:65, 1, 1:1 + (NT - 1) * PT],
                              in_=packT[0:NT - 1, :])
            nc.sync.dma_start(out=xt[64:65, 1, 1 + (NT - 1) * PT:1 + L],
                              in_=packT[NT - 1:NT, 0:TAIL])
            rrow = stat.tile([1, L], BF16)
            nc.sync.dma_start(out=rrow[0:1, 0:(NT - 1) * PT],
                              in_=packT[64:64 + NT - 1, :])
            nc.sync.dma_start(out=rrow[0:1, (NT - 1) * PT:L],
                              in_=packT[64 + NT - 1:64 + NT, 0:TAIL])
            rstd_b = rbpool.tile([128, L], BF16)
            nc.gpsimd.partition_broadcast(rstd_b, rrow[0:1, :])

            # ---- fold rstd into the fc1 rhs: one column scale of both
            #      channel planes (guard cols 0 and L+1 stay zero)
            nc.vector.tensor_tensor(out=xt[:, 0, 1:1 + L], in0=xt[:, 0, 1:1 + L],
                                    in1=rstd_b, op=AL.mult)
            nc.vector.tensor_tensor(out=xt[:, 1, 1:1 + L], in0=xt[:, 1, 1:1 + L],
                                    in1=rstd_b, op=AL.mult)
            state[s] = xt

        def main_sample(s):
            xt = state.pop(s)

            # ---- fc1 + depthwise conv fused on PE: 3 shifted tap matmuls
            #      (x2 K-chunks) accumulate conv(u) per block; gelu (with
            #      deferred biases) evacuates PSUM directly.
            t = tpool.tile([128, 3, L], BF16)
            for blk in range(NBLK):
                cs = blk * NB
                for m in range(3):
                    pt_ = pf1.tile([128, NB], F32)
                    for k in range(3):
                        nc.tensor.matmul(pt_,
                                         lhsT=w1a[:, k, m * 128:(m + 1) * 128],
                                         rhs=xt[:, 0, cs + k:cs + k + NB],
                                         start=(k == 0), stop=False)
                        nc.tensor.matmul(pt_,
                                         lhsT=w1b[:, k, m * 128:(m + 1) * 128],
                                         rhs=xt[0:65, 1, cs + k:cs + k + NB],
                                         start=False, stop=(k == 2))
                    nc.scalar.activation(out=t[:, m, cs:cs + NB], in_=pt_,
                                         func=AF.Gelu, bias=gb3[:, m, 0:1],
                                         scale=1.0)
                    if blk == 0:
                        nc.scalar.activation(out=t[:, m, 0:1], in_=pt_[:, 0:1],
                                             func=AF.Gelu, bias=gb3[:, m, 1:2],
                                             scale=1.0)
                    if blk == NBLK - 1:
                        nc.scalar.activation(out=t[:, m, L - 1:L],
                                             in_=pt_[:, NB - 1:NB],
                                             func=AF.Gelu, bias=gb3[:, m, 2:3],
                                             scale=1.0)

            # ---- fc2 channel-major: w2 chunks stationary, t moving; emits
            #      F = t @ W2 (bias + residual live on the host)
            f_sb = fpool.tile([128, 2, L], BF16)
            for blk in range(NBLK):
                cs = blk * NB
                p0 = pf2.tile([128, NB], F32, tag="p0")
                p1 = pf2.tile([64, NB], F32, tag="p1")
                for d3 in range(3):
                    nc.tensor.matmul(p0, lhsT=w2[:, d3, 0:128],
                                     rhs=t[:, d3, cs:cs + NB],
                                     start=(d3 == 0), stop=(d3 == 2))
                for d3 in range(3):
                    nc.tensor.matmul(p1, lhsT=w2[:, d3, 128:192],
                                     rhs=t[:, d3, cs:cs + NB],
                                     start=(d3 == 0), stop=(d3 == 2))
                nc.scalar.copy(out=f_sb[:, 0, cs:cs + NB], in_=p0)
                nc.scalar.copy(out=f_sb[0:64, 1, cs:cs + NB], in_=p1)
            nc.sync.dma_start(out=f_d[s, :, :, :], in_=f_sb)

        # stats/rstd prep of sample s+1 is emitted ahead of sample s's heavy
        # phases so the stats chain never gates the next sample's fc1
        pre_sample(0)
        for s in range(S):
            if s + 1 < S:
                pre_sample(s + 1)
            main_sample(s)
    nc.finalize()
    return nc


def _get_nc():
    if "nc" not in _CACHE:
        _CACHE["nc"] = _build_nc()
    return _CACHE["nc"]


def _host_params(gamma, beta, fc1_w, fc1_b, dw_w, dw_b, fc2_w, fc2_b):
    bf = ml_dtypes.bfloat16
    w1g = (fc1_w * gamma[:, None]).astype(np.float32)          # [192, 384]
    s1g = w1g.sum(0)                                           # [384]
    b1aug = (beta @ fc1_w + fc1_b).astype(np.float32)          # [384]
    dwtaps = dw_w[:, 0, :].T.astype(np.float32)                # [3, 384]
    w1a = (w1g[0:128][:, None, :] * dwtaps[None, :, :]).astype(bf)
    w1b_base = np.concatenate([w1g[128:192], -s1g[None, :]], 0)  # [65, 384]
    w1b = (w1b_base[:, None, :] * dwtaps[None, :, :]).astype(bf)
    w2 = np.ascontiguousarray(
        fc2_w.reshape(3, 128, C).transpose(1, 0, 2)).astype(bf)  # [128,3,192]
    w0, w1_, w2_ = dwtaps[0], dwtaps[1], dwtaps[2]
    gb_int = dw_b + b1aug * (w0 + w1_ + w2_)
    gb_l = dw_b + b1aug * (w1_ + w2_)        # col 0: tap0 falls on zero pad
    gb_r = dw_b + b1aug * (w0 + w1_)         # col L-1: tap2 falls on zero pad
    gb3 = np.ascontiguousarray(
        np.stack([gb_int, gb_l, gb_r], -1).reshape(3, 128, 3)
        .transpose(1, 0, 2)).astype(np.float32)                # [128, 3, 3]
    return dict(w1a=np.ascontiguousarray(w1a),
                w1b=np.ascontiguousarray(w1b), w2=w2, gb3=gb3)


def _host_xt(x_dev):
    """Channel-major bf16 copy of x: [B, 128, 2, L+2] with zero guard columns
    at 0 and L+1 (the conv's zero padding); [:, 64:, 1, :] holds the runtime
    mu row (slot 64) and padding, zero-filled here."""
    bf = ml_dtypes.bfloat16
    nb = x_dev.shape[0]
    arr = np.ascontiguousarray(
        x_dev.reshape(nb, L, C).transpose(0, 2, 1)).astype(bf)  # [nb, 192, L]
    xt = np.zeros((nb, 128, 2, L + 2), dtype=bf)
    xt[:, :, 0, 1:1 + L] = arr[:, 0:128]
    xt[:, 0:64, 1, 1:1 + L] = arr[:, 128:192]
    return xt


def _selector_flags(x, gamma, beta, sel_w1, sel_b1, sel_w2, sel_b2):
    """Exact numpy replica of the reference direction selector. Only used
    when gamma is non-uniform (otherwise the scores tie and idx==0 always)."""
    xf = x.astype(np.float32)
    mu = xf.mean(-1, keepdims=True)
    var = ((xf - mu) ** 2).mean(-1, keepdims=True)
    xn = (xf - mu) / np.sqrt(var + EPS) * gamma + beta
    xg = xn.mean(-1)
    gh = np.abs(xg[:, :, 1:] - xg[:, :, :-1]).mean(axis=(1, 2))
    gv = np.abs(xg[:, 1:, :] - xg[:, :-1, :]).mean(axis=(1, 2))
    scores = np.stack([gh, gv, 0.8 * (gh + gv) * 0.5, np.abs(gh - gv)], 1)
    hdn = np.maximum(scores @ sel_w1 + sel_b1, 0.0)
    logits = hdn @ sel_w2 + sel_b2
    ex = np.exp(logits - logits.max(1, keepdims=True))
    probs = ex / ex.sum(1, keepdims=True)
    return probs.argmax(1) % 4 == 1


def build_in_maps(inputs):
    """Shared by kernel() and test harnesses: host preprocessing + sharding.
    Returns (in_maps, x, x_dev, flags)."""
    bf = ml_dtypes.bfloat16
    x = np.asarray(inputs["x"], dtype=np.float32)
    gamma = np.asarray(inputs["gamma"], np.float32)
    beta = np.asarray(inputs["beta"], np.float32)
    params = _host_params(
        gamma, beta,
        np.asarray(inputs["fc1_w"], np.float32),
        np.asarray(inputs["fc1_b"], np.float32),
        np.asarray(inputs["dw_w"], np.float32),
        np.asarray(inputs["dw_b"], np.float32),
        np.asarray(inputs["fc2_w"], np.float32),
        np.asarray(inputs["fc2_b"], np.float32),
    )

    # Routing: uniform gamma => gray image is constant => scores tie => idx 0
    # for every sample (see module docstring).  Otherwise compute the selector
    # on host and pre-transpose flagged samples (mathematically exact fixup).
    if np.ptp(gamma) == 0.0:
        flags = np.zeros(B, dtype=bool)
    else:
        flags = _selector_flags(
            x, gamma, beta,
            np.asarray(inputs["sel_w1"], np.float32),
            np.asarray(inputs["sel_b1"], np.float32),
            np.asarray(inputs["sel_w2"], np.float32),
            np.asarray(inputs["sel_b2"], np.float32))
    x_dev = x
    if flags.any():
        x_dev = x.copy()
        x_dev[flags] = np.swapaxes(x_dev[flags], 1, 2)

    xt = _host_xt(x_dev)
    xs = x_dev.reshape(B, L, C).astype(bf)
    in_maps = []
    for i in range(NCORES):
        m = {"xs": np.ascontiguousarray(
                 xs[S * i:S * (i + 1)].reshape(S * L, C)),
             "xt": xt[S * i:S * (i + 1)]}
        m.update(params)
        in_maps.append(m)
    return in_maps, x, x_dev, flags


def kernel(**inputs):
    from concourse.bass_utils import run_bass_kernel_spmd

    in_maps, x, x_dev, flags = build_in_maps(inputs)
    nc = _get_nc()
    res = run_bass_kernel_spmd(nc, in_maps, list(range(NCORES)))
    fcm = np.empty((B, 192, L), np.float32)
    for i, r in enumerate(res.results):
        fcm[S * i:S * (i + 1), 0:128] = r["F"][:, :, 0, :]
        fcm[S * i:S * (i + 1), 128:192] = r["F"][:, 0:64, 1, :]
    out = fcm.transpose(0, 2, 1).reshape(B, H, W, C)
    fc2_b = np.asarray(inputs["fc2_b"], np.float32)
    # device computed F(x_dev); reference wants x + F(x_dev) + fc2_b
    # (row-major unscan orientation is identical for flagged samples)
    y = x + out + fc2_b
    return y.astype(np.float32)


# revision 11
# speedup vs baseline: 1.4502x; 1.0289x over previous
"""CASS block (LayerNorm + gradient-selected scan + fc1/dwconv/gelu/fc2 + residual)
on 8 TRN2 NeuronCores, pure data parallel over the batch.

v3 — PE-centric restructure.  Key algebra: with per-pixel LN stats (mu, rstd),
    u = LN(x) @ W1 + b1 = (rstd .* [x; mu]) @ [W1g; -s1g] + b1aug
so the per-pixel rstd is folded into the matmul *rhs* (column scale of the
channel-major input) instead of the psum output (3x larger).  The 3-tap
depthwise conv that follows fc1 is linear, so it is folded into the fc1
matmul itself: conv(u)[l] = sum_k (W1g .* w_k)^T xr[l+k-1], i.e. three
shifted accumulating matmuls per output block against one zero-guarded
channel-major input (guard columns give the conv's zero padding for free).
Gelu (with the fc1/dwconv biases deferred into its bias operand, plus 2
boundary-column fixups) reads the accumulated PSUM directly.  fc2 runs
channel-major with w2 as the stationary operand, and the kernel emits only
the block output F = fc2(gelu(...)); the residual y = x + F + fc2_b is an
elementwise epilogue done host-side in fp32 (exact x, no back-transpose).

Latency details: the rstd row is broadcast across partitions with a K=1
ones-matmul into PSUM (per 392-col block) that the DVE column-scale reads
directly — no gpsimd.  A burst of throwaway matmuls at kernel start keeps
the PE busy while sample 0's stats chain runs, so the HAM clock-gate is
warm (2.4 GHz) when the real fc1 stream begins.  fc1/fc2 are interleaved
per half-image so the output DMA of half 0 overlaps compute of half 1.

The gradient selector: for uniform gamma the "gray" image mean_c(LN(x)) is a
constant, so grad_h = grad_v = 0, the MLP logits tie, softmax gives exactly
0.25 each in fp32, and argmax -> idx 0 for every sample: the 'v' (transpose)
branch is dead.  The device kernel therefore always scans row-major; a host
fallback handles non-uniform gamma by pre-transposing flagged samples (the
row-major reshape of the result is orientation-identical, so y = x + F(x_dev)
recovers the reference output exactly).
"""

import numpy as np
import ml_dtypes

import concourse.mybir as mybir
import concourse.tile as tile
from concourse import bacc

B, H, W, C = 32, 56, 56, 192
D = 384                      # D_INNER
NCORES = 8
S = B // NCORES              # samples per core
L = H * W                    # 3136 pixels per sample
PT = 128                     # pixels per partition tile (stats)
NT = (L + PT - 1) // PT      # 25 pixel tiles (24 full + 64 tail)
TAIL = L - (NT - 1) * PT     # 64
NB = 448                     # columns per matmul block
NBLK = L // NB               # 7 blocks
NWARM = 104                  # HAM warmup matmuls
EPS = 1e-5
F32 = mybir.dt.float32
BF16 = mybir.dt.bfloat16
AL = mybir.AluOpType
AF = mybir.ActivationFunctionType

_CACHE = {}


def _build_nc():
    nc = bacc.Bacc()
    xs_d = nc.declare_dram_parameter("xs", [S * L, C], BF16, isOutput=False)
    xt_d = nc.declare_dram_parameter("xt", [S, 128, 2, L + 2], BF16,
                                     isOutput=False)
    w1a_d = nc.declare_dram_parameter("w1a", [128, 3, D], BF16, isOutput=False)
    w1b_d = nc.declare_dram_parameter("w1b", [65, 3, D], BF16, isOutput=False)
    w2_d = nc.declare_dram_parameter("w2", [128, 3, C], BF16, isOutput=False)
    gb3_d = nc.declare_dram_parameter("gb3", [128, 3, 3], F32, isOutput=False)
    f_d = nc.declare_dram_parameter("F", [S, 128, 2, L], BF16, isOutput=True)

    with tile.TileContext(nc) as tc, \
         tc.tile_pool(name="const", bufs=1) as const, \
         tc.tile_pool(name="xs", bufs=2) as xspool, \
         tc.tile_pool(name="xt", bufs=2) as xtpool, \
         tc.tile_pool(name="stat", bufs=2) as stat, \
         tc.tile_pool(name="t", bufs=2) as tpool, \
         tc.tile_pool(name="f", bufs=2) as fpool, \
         tc.tile_pool(name="pf1", bufs=4, space="PSUM") as pf1, \
         tc.tile_pool(name="rbp", bufs=2, space="PSUM") as rbp, \
         tc.tile_pool(name="pf2", bufs=1, space="PSUM") as pf2:

        # ---- HAM warmup: keep the PE busy from t~5us while sample 0's
        #      stats chain runs, so fc1 starts at 2.4 GHz.  Junk results
        #      rotate through the rb slot and are never read.
        junk = const.tile([128, NB], BF16)
        nc.vector.memset(junk, 0.0)
        for _ in range(NWARM):
            jp = rbp.tile([128, NB], F32, name="rb", tag="rb")
            nc.tensor.matmul(jp, lhsT=junk[:, 0:128], rhs=junk,
                             start=True, stop=True)

        w1a = const.tile([128, 3, D], BF16)
        w1b = const.tile([65, 3, D], BF16)
        w2 = const.tile([128, 3, C], BF16)
        gb3 = const.tile([128, 3, 3], F32)
        eps_sb = const.tile([128, 1], F32)
        ones = const.tile([1, 128], BF16)
        nc.sync.dma_start(out=w1a, in_=w1a_d[:, :, :])
        nc.sync.dma_start(out=w1b, in_=w1b_d[:, :, :])
        nc.sync.dma_start(out=w2, in_=w2_d[:, :, :])
        nc.sync.dma_start(out=gb3, in_=gb3_d[:, :, :])
        nc.vector.memset(eps_sb, EPS)
        nc.vector.memset(ones, 1.0)

        state = {}

        def pre_sample(s):
            base = s * L

            # ---- pixel-major bf16 x (stats only) + LN stats
            #      pack[:,0,k]=mu_k, pack[:,1,k]=var_k->rstd_k
            bns = stat.tile([128, NT, 6], F32)
            pack = stat.tile([128, 2, 64], F32)
            packb = stat.tile([128, 2, 64], BF16)
            nc.vector.memset(pack, 0.0)
            xs_sb = xspool.tile([128, NT, C], BF16)
            for j in range(12):
                nc.sync.dma_start(
                    out=xs_sb[:, 2 * j:2 * j + 2, :],
                    in_=xs_d[base + j * 256: base + (j + 1) * 256, :]
                        .rearrange("(two p) c -> p two c", p=128),
                )
            nc.sync.dma_start(
                out=xs_sb[0:TAIL, NT - 1, :],
                in_=xs_d[base + (NT - 1) * PT: base + L, :],
            )
            xt = xtpool.tile([128, 2, L + 2], BF16)
            nc.sync.dma_start(out=xt, in_=xt_d[s, :, :, :])
            for k in range(NT - 1):
                nc.vector.bn_stats(out=bns[:, k:k + 1, :],
                                   in_=xs_sb[:, k:k + 1, :])
                nc.vector.bn_aggr(out=pack[:, :, k], in_=bns[:, k:k + 1, :])
            nc.vector.bn_stats(out=bns[0:TAIL, NT - 1:NT, :],
                               in_=xs_sb[0:TAIL, NT - 1:NT, :])
            nc.vector.bn_aggr(out=pack[0:TAIL, :, NT - 1],
                              in_=bns[0:TAIL, NT - 1:NT, :])
            # rstd = 1/sqrt(var+eps) in place
            nc.scalar.activation(out=pack[:, 1, 0:NT], in_=pack[:, 1, 0:NT],
                                 func=AF.Sqrt, bias=eps_sb[:, :], scale=1.0)
            nc.vector.reciprocal(out=pack[:, 1, 0:NT], in_=pack[:, 1, 0:NT])
            nc.vector.tensor_copy(out=packb, in_=pack)

            # ---- one small xbar transpose + row-linearize DMAs:
            #      mu row -> xt[64,1,:] (fc1 aug row), rstd -> rrow
            packT = stat.tile([128, 128], BF16)
            nc.sync.dma_start(out=packT,
                              in_=packb.rearrange("p a b -> p (a b)"),
                              transpose=True)
            nc.sync.dma_start(out=xt[64:65, 1, 1:1 + (NT - 1) * PT],
                              in_=packT[0:NT - 1, :])
            nc.sync.dma_start(out=xt[64:65, 1, 1 + (NT - 1) * PT:1 + L],
                              in_=packT[NT - 1:NT, 0:TAIL])
            rrow = stat.tile([1, L], BF16)
            nc.sync.dma_start(out=rrow[0:1, 0:(NT - 1) * PT],
                              in_=packT[64:64 + NT - 1, :])
            nc.sync.dma_start(out=rrow[0:1, (NT - 1) * PT:L],
                              in_=packT[64 + NT - 1:64 + NT, 0:TAIL])
            state[s] = (xt, rrow)

        def main_sample(s):
            xt, rrow = state.pop(s)

            # ---- fold rstd into the fc1 rhs: per block, broadcast the rstd
            #      row across partitions with a K=1 ones-matmul into PSUM and
            #      column-scale both channel planes from it (guards stay 0).
            for blk in range(NBLK):
                js = blk * NB
                rb = rbp.tile([128, NB], F32, name="rb", tag="rb")
                nc.tensor.matmul(rb, lhsT=ones[0:1, :],
                                 rhs=rrow[0:1, js:js + NB],
                                 start=True, stop=True)
                nc.vector.tensor_tensor(out=xt[:, 0, 1 + js:1 + js + NB],
                                        in0=xt[:, 0, 1 + js:1 + js + NB],
                                        in1=rb, op=AL.mult)
                nc.vector.tensor_tensor(out=xt[:, 1, 1 + js:1 + js + NB],
                                        in0=xt[:, 1, 1 + js:1 + js + NB],
                                        in1=rb, op=AL.mult)

            # ---- fc1 + conv fused on PE: 3 shifted tap matmuls x2 K-chunks
            #      accumulate conv(u) per block; gelu (deferred biases)
            #      evacuates PSUM directly.
            t = tpool.tile([128, 3, L], BF16)
            for blk in range(NBLK):
                cs = blk * NB
                for m in range(3):
                    pt_ = pf1.tile([128, NB], F32)
                    for k in range(3):
                        nc.tensor.matmul(pt_,
                                         lhsT=w1a[:, k, m * 128:(m + 1) * 128],
                                         rhs=xt[:, 0, cs + k:cs + k + NB],
                                         start=(k == 0), stop=False)
                        nc.tensor.matmul(pt_,
                                         lhsT=w1b[:, k, m * 128:(m + 1) * 128],
                                         rhs=xt[0:65, 1, cs + k:cs + k + NB],
                                         start=False, stop=(k == 2))
                    nc.scalar.activation(out=t[:, m, cs:cs + NB], in_=pt_,
                                         func=AF.Gelu, bias=gb3[:, m, 0:1],
                                         scale=1.0)
                    if blk == 0:
                        nc.scalar.activation(out=t[:, m, 0:1], in_=pt_[:, 0:1],
                                             func=AF.Gelu, bias=gb3[:, m, 1:2],
                                             scale=1.0)
                    if blk == NBLK - 1:
                        nc.scalar.activation(out=t[:, m, L - 1:L],
                                             in_=pt_[:, NB - 1:NB],
                                             func=AF.Gelu, bias=gb3[:, m, 2:3],
                                             scale=1.0)

            # ---- fc2 channel-major: w2 chunks stationary, t moving; emits
            #      F = t @ W2 (bias + residual live on the host).  The
            #      first 4 blocks' output DMA overlaps the tail compute.
            f_sb = fpool.tile([128, 2, L], BF16)
            for blk in range(NBLK):
                cs = blk * NB
                p0 = pf2.tile([128, NB], F32, tag="p0")
                p1 = pf2.tile([64, NB], F32, tag="p1")
                for d3 in range(3):
                    nc.tensor.matmul(p0, lhsT=w2[:, d3, 0:128],
                                     rhs=t[:, d3, cs:cs + NB],
                                     start=(d3 == 0), stop=(d3 == 2))
                for d3 in range(3):
                    nc.tensor.matmul(p1, lhsT=w2[:, d3, 128:192],
                                     rhs=t[:, d3, cs:cs + NB],
                                     start=(d3 == 0), stop=(d3 == 2))
                nc.scalar.copy(out=f_sb[:, 0, cs:cs + NB], in_=p0)
                nc.scalar.copy(out=f_sb[0:64, 1, cs:cs + NB], in_=p1)
                if blk == 3:
                    nc.sync.dma_start(out=f_d[s, :, :, 0:4 * NB],
                                      in_=f_sb[:, :, 0:4 * NB])
            nc.sync.dma_start(out=f_d[s, :, :, 4 * NB:L],
                              in_=f_sb[:, :, 4 * NB:L])

        # stats of sample s+1 run on DVE behind sample s's column scales,
        # inside sample s's PE window
        pre_sample(0)
        for s in range(S):
            main_sample(s)
            if s + 1 < S:
                pre_sample(s + 1)
    nc.finalize()
    return nc


def _get_nc():
    if "nc" not in _CACHE:
        _CACHE["nc"] = _build_nc()
    return _CACHE["nc"]


def _host_params(gamma, beta, fc1_w, fc1_b, dw_w, dw_b, fc2_w, fc2_b):
    bf = ml_dtypes.bfloat16
    w1g = (fc1_w * gamma[:, None]).astype(np.float32)          # [192, 384]
    s1g = w1g.sum(0)                                           # [384]
    b1aug = (beta @ fc1_w + fc1_b).astype(np.float32)          # [384]
    dwtaps = dw_w[:, 0, :].T.astype(np.float32)                # [3, 384]
    w1a = (w1g[0:128][:, None, :] * dwtaps[None, :, :]).astype(bf)
    w1b_base = np.concatenate([w1g[128:192], -s1g[None, :]], 0)  # [65, 384]
    w1b = (w1b_base[:, None, :] * dwtaps[None, :, :]).astype(bf)
    w2 = np.ascontiguousarray(
        fc2_w.reshape(3, 128, C).transpose(1, 0, 2)).astype(bf)  # [128,3,192]
    w0, w1_, w2_ = dwtaps[0], dwtaps[1], dwtaps[2]
    gb_int = dw_b + b1aug * (w0 + w1_ + w2_)
    gb_l = dw_b + b1aug * (w1_ + w2_)        # col 0: tap0 falls on zero pad
    gb_r = dw_b + b1aug * (w0 + w1_)         # col L-1: tap2 falls on zero pad
    gb3 = np.ascontiguousarray(
        np.stack([gb_int, gb_l, gb_r], -1).reshape(3, 128, 3)
        .transpose(1, 0, 2)).astype(np.float32)                # [128, 3, 3]
    return dict(w1a=np.ascontiguousarray(w1a),
                w1b=np.ascontiguousarray(w1b), w2=w2, gb3=gb3)


def _host_xt(x_dev):
    """Channel-major bf16 copy of x: [B, 128, 2, L+2] with zero guard columns
    at 0 and L+1 (the conv's zero padding); [:, 64:, 1, :] holds the runtime
    mu row (slot 64) and padding, zero-filled here."""
    bf = ml_dtypes.bfloat16
    nb = x_dev.shape[0]
    arr = np.ascontiguousarray(
        x_dev.reshape(nb, L, C).transpose(0, 2, 1)).astype(bf)  # [nb, 192, L]
    xt = np.zeros((nb, 128, 2, L + 2), dtype=bf)
    xt[:, :, 0, 1:1 + L] = arr[:, 0:128]
    xt[:, 0:64, 1, 1:1 + L] = arr[:, 128:192]
    return xt


def _selector_flags(x, gamma, beta, sel_w1, sel_b1, sel_w2, sel_b2):
    """Exact numpy replica of the reference direction selector. Only used
    when gamma is non-uniform (otherwise the scores tie and idx==0 always)."""
    xf = x.astype(np.float32)
    mu = xf.mean(-1, keepdims=True)
    var = ((xf - mu) ** 2).mean(-1, keepdims=True)
    xn = (xf - mu) / np.sqrt(var + EPS) * gamma + beta
    xg = xn.mean(-1)
    gh = np.abs(xg[:, :, 1:] - xg[:, :, :-1]).mean(axis=(1, 2))
    gv = np.abs(xg[:, 1:, :] - xg[:, :-1, :]).mean(axis=(1, 2))
    scores = np.stack([gh, gv, 0.8 * (gh + gv) * 0.5, np.abs(gh - gv)], 1)
    hdn = np.maximum(scores @ sel_w1 + sel_b1, 0.0)
    logits = hdn @ sel_w2 + sel_b2
    ex = np.exp(logits - logits.max(1, keepdims=True))
    probs = ex / ex.sum(1, keepdims=True)
    return probs.argmax(1) % 4 == 1


def build_in_maps(inputs):
    """Shared by kernel() and test harnesses: host preprocessing + sharding.
    Returns (in_maps, x, x_dev, flags)."""
    bf = ml_dtypes.bfloat16
    x = np.asarray(inputs["x"], dtype=np.float32)
    gamma = np.asarray(inputs["gamma"], np.float32)
    beta = np.asarray(inputs["beta"], np.float32)
    params = _host_params(
        gamma, beta,
        np.asarray(inputs["fc1_w"], np.float32),
        np.asarray(inputs["fc1_b"], np.float32),
        np.asarray(inputs["dw_w"], np.float32),
        np.asarray(inputs["dw_b"], np.float32),
        np.asarray(inputs["fc2_w"], np.float32),
        np.asarray(inputs["fc2_b"], np.float32),
    )

    # Routing: uniform gamma => gray image is constant => scores tie => idx 0
    # for every sample (see module docstring).  Otherwise compute the selector
    # on host and pre-transpose flagged samples (mathematically exact fixup).
    if np.ptp(gamma) == 0.0:
        flags = np.zeros(B, dtype=bool)
    else:
        flags = _selector_flags(
            x, gamma, beta,
            np.asarray(inputs["sel_w1"], np.float32),
            np.asarray(inputs["sel_b1"], np.float32),
            np.asarray(inputs["sel_w2"], np.float32),
            np.asarray(inputs["sel_b2"], np.float32))
    x_dev = x
    if flags.any():
        x_dev = x.copy()
        x_dev[flags] = np.swapaxes(x_dev[flags], 1, 2)

    xt = _host_xt(x_dev)
    xs = x_dev.reshape(B, L, C).astype(bf)
    in_maps = []
    for i in range(NCORES):
        m = {"xs": np.ascontiguousarray(
                 xs[S * i:S * (i + 1)].reshape(S * L, C)),
             "xt": xt[S * i:S * (i + 1)]}
        m.update(params)
        in_maps.append(m)
    return in_maps, x, x_dev, flags


def kernel(**inputs):
    from concourse.bass_utils import run_bass_kernel_spmd

    in_maps, x, x_dev, flags = build_in_maps(inputs)
    nc = _get_nc()
    res = run_bass_kernel_spmd(nc, in_maps, list(range(NCORES)))
    fcm = np.empty((B, 192, L), np.float32)
    for i, r in enumerate(res.results):
        fcm[S * i:S * (i + 1), 0:128] = r["F"][:, :, 0, :]
        fcm[S * i:S * (i + 1), 128:192] = r["F"][:, 0:64, 1, :]
    out = fcm.transpose(0, 2, 1).reshape(B, H, W, C)
    fc2_b = np.asarray(inputs["fc2_b"], np.float32)
    # device computed F(x_dev); reference wants x + F(x_dev) + fc2_b
    # (row-major unscan orientation is identical for flagged samples)
    y = x + out + fc2_b
    return y.astype(np.float32)


# revision 17
# speedup vs baseline: 1.5673x; 1.0807x over previous
"""CASS block (LayerNorm + gradient-selected scan + fc1/dwconv/gelu/fc2 + residual)
on 8 TRN2 NeuronCores, pure data parallel over the batch.

v3 — PE-centric restructure.  Key algebra: with per-pixel LN stats (mu, rstd),
    u = LN(x) @ W1 + b1 = (rstd .* [x; mu]) @ [W1g; -s1g] + b1aug
so the per-pixel rstd is folded into the matmul *rhs* (column scale of the
channel-major input) instead of the psum output (3x larger).  The 3-tap
depthwise conv that follows fc1 is linear, so it is folded into the fc1
matmul itself: conv(u)[l] = sum_k (W1g .* w_k)^T xr[l+k-1], i.e. three
shifted accumulating matmuls per output block against one zero-guarded
channel-major input (guard columns give the conv's zero padding for free).
Gelu (with the fc1/dwconv biases deferred into its bias operand, plus 2
boundary-column fixups) reads the accumulated PSUM directly.  fc2 runs
channel-major with w2 as the stationary operand, and the kernel emits only
the block output F = fc2(gelu(...)); the residual y = x + F + fc2_b is an
elementwise epilogue done host-side in fp32 (exact x, no back-transpose).

Latency details: the rstd row is broadcast across partitions with a K=1
ones-matmul into PSUM (per 392-col block) that the DVE column-scale reads
directly — no gpsimd.  A burst of throwaway matmuls at kernel start keeps
the PE busy while sample 0's stats chain runs, so the HAM clock-gate is
warm (2.4 GHz) when the real fc1 stream begins.  fc1/fc2 are interleaved
per half-image so the output DMA of half 0 overlaps compute of half 1.

The gradient selector: for uniform gamma the "gray" image mean_c(LN(x)) is a
constant, so grad_h = grad_v = 0, the MLP logits tie, softmax gives exactly
0.25 each in fp32, and argmax -> idx 0 for every sample: the 'v' (transpose)
branch is dead.  The device kernel therefore always scans row-major; a host
fallback handles non-uniform gamma by pre-transposing flagged samples (the
row-major reshape of the result is orientation-identical, so y = x + F(x_dev)
recovers the reference output exactly).
"""

import numpy as np
import ml_dtypes

import concourse.mybir as mybir
import concourse.tile as tile
from concourse import bacc

B, H, W, C = 32, 56, 56, 192
D = 384                      # D_INNER
NCORES = 8
S = B // NCORES              # samples per core
L = H * W                    # 3136 pixels per sample
PT = 128                     # pixels per partition tile (stats)
NT = (L + PT - 1) // PT      # 25 pixel tiles (24 full + 64 tail)
TAIL = L - (NT - 1) * PT     # 64
NB = 448                     # columns per matmul block
NBLK = L // NB               # 7 blocks
NWARM = 152                  # HAM warmup matmuls
EPS = 1e-5
F32 = mybir.dt.float32
BF16 = mybir.dt.bfloat16
AL = mybir.AluOpType
AF = mybir.ActivationFunctionType

_CACHE = {}


def _build_nc():
    nc = bacc.Bacc()
    xs_d = nc.declare_dram_parameter("xs", [S * L, C], BF16, isOutput=False)
    xt_d = nc.declare_dram_parameter("xt", [S, 128, 2, L + 2], BF16,
                                     isOutput=False)
    w1a_d = nc.declare_dram_parameter("w1a", [128, 3, D], BF16, isOutput=False)
    w1b_d = nc.declare_dram_parameter("w1b", [65, 3, D], BF16, isOutput=False)
    w2_d = nc.declare_dram_parameter("w2", [128, 3, C], BF16, isOutput=False)
    gb3_d = nc.declare_dram_parameter("gb3", [128, 3, 3], F32, isOutput=False)
    f_d = nc.declare_dram_parameter("F", [S, 128, 2, L], BF16, isOutput=True)

    with tile.TileContext(nc) as tc, \
         tc.tile_pool(name="const", bufs=1) as const, \
         tc.tile_pool(name="xs", bufs=2) as xspool, \
         tc.tile_pool(name="xt", bufs=2) as xtpool, \
         tc.tile_pool(name="stat", bufs=2) as stat, \
         tc.tile_pool(name="t", bufs=2) as tpool, \
         tc.tile_pool(name="f", bufs=2) as fpool, \
         tc.tile_pool(name="rb", bufs=2) as rbpool, \
         tc.tile_pool(name="pf1", bufs=4, space="PSUM") as pf1, \
         tc.tile_pool(name="pf2", bufs=2, space="PSUM") as pf2:

        # ---- HAM warmup: keep the PE busy from t~5us until sample 0's
        #      stats chain delivers real work, so fc1 starts at 2.4 GHz
        #      (an idle gap here re-throttles the clock for ~20us).  Junk
        #      results rotate through the fc1 psum slot, never read.
        junk = const.tile([128, NB], BF16)
        nc.vector.memset(junk, 0.0)
        for _ in range(NWARM):
            jp = pf1.tile([128, NB], F32, name="pt_", tag="pt_")
            nc.tensor.matmul(jp, lhsT=junk[:, 0:128], rhs=junk,
                             start=True, stop=True)

        w1a = const.tile([128, 3, D], BF16)
        w1b = const.tile([65, 3, D], BF16)
        w2 = const.tile([128, 3, C], BF16)
        gb3 = const.tile([128, 3, 3], F32)
        eps_sb = const.tile([128, 1], F32)
        nc.sync.dma_start(out=w1a, in_=w1a_d[:, :, :])
        nc.sync.dma_start(out=w1b, in_=w1b_d[:, :, :])
        nc.sync.dma_start(out=w2, in_=w2_d[:, :, :])
        nc.sync.dma_start(out=gb3, in_=gb3_d[:, :, :])
        nc.vector.memset(eps_sb, EPS)

        state = {}

        def pre_sample(s):
            base = s * L

            # ---- pixel-major bf16 x (stats only) + LN stats
            #      pack[:,0,k]=mu_k, pack[:,1,k]=var_k->rstd_k
            bns = stat.tile([128, NT, 6], F32)
            pack = stat.tile([128, 2, 64], F32)
            packb = stat.tile([128, 2, 64], BF16)
            nc.vector.memset(pack, 0.0)
            xs_sb = xspool.tile([128, NT, C], BF16)
            for j in range(12):
                nc.sync.dma_start(
                    out=xs_sb[:, 2 * j:2 * j + 2, :],
                    in_=xs_d[base + j * 256: base + (j + 1) * 256, :]
                        .rearrange("(two p) c -> p two c", p=128),
                )
            nc.sync.dma_start(
                out=xs_sb[0:TAIL, NT - 1, :],
                in_=xs_d[base + (NT - 1) * PT: base + L, :],
            )
            xt = xtpool.tile([128, 2, L + 2], BF16)
            nc.sync.dma_start(out=xt, in_=xt_d[s, :, :, :])
            for k in range(NT - 1):
                nc.vector.bn_stats(out=bns[:, k:k + 1, :],
                                   in_=xs_sb[:, k:k + 1, :])
                nc.vector.bn_aggr(out=pack[:, :, k], in_=bns[:, k:k + 1, :])
            nc.vector.bn_stats(out=bns[0:TAIL, NT - 1:NT, :],
                               in_=xs_sb[0:TAIL, NT - 1:NT, :])
            nc.vector.bn_aggr(out=pack[0:TAIL, :, NT - 1],
                              in_=bns[0:TAIL, NT - 1:NT, :])
            # rstd = 1/sqrt(var+eps) in place
            nc.scalar.activation(out=pack[:, 1, 0:NT], in_=pack[:, 1, 0:NT],
                                 func=AF.Sqrt, bias=eps_sb[:, :], scale=1.0)
            nc.vector.reciprocal(out=pack[:, 1, 0:NT], in_=pack[:, 1, 0:NT])
            nc.vector.tensor_copy(out=packb, in_=pack)

            # ---- one small xbar transpose + row-linearize DMAs:
            #      mu row -> xt[64,1,:] (fc1 aug row), rstd -> rrow
            packT = stat.tile([128, 128], BF16)
            nc.sync.dma_start(out=packT,
                              in_=packb.rearrange("p a b -> p (a b)"),
                              transpose=True)
            nc.sync.dma_start(out=xt[64:65, 1, 1:1 + (NT - 1) * PT],
                              in_=packT[0:NT - 1, :])
            nc.sync.dma_start(out=xt[64:65, 1, 1 + (NT - 1) * PT:1 + L],
                              in_=packT[NT - 1:NT, 0:TAIL])
            rrow = stat.tile([1, L], BF16)
            nc.sync.dma_start(out=rrow[0:1, 0:(NT - 1) * PT],
                              in_=packT[64:64 + NT - 1, :])
            nc.sync.dma_start(out=rrow[0:1, (NT - 1) * PT:L],
                              in_=packT[64 + NT - 1:64 + NT, 0:TAIL])
            state[s] = (xt, rrow)

        def main_sample(s):
            xt, rrow = state.pop(s)

            # ---- fold rstd into the fc1 rhs: per block, broadcast the rstd
            #      row across partitions (gpsimd, off every critical engine)
            #      and column-scale both channel planes (DVE 2x, bf16).
            rstd_b = rbpool.tile([128, L], BF16)
            for blk in range(NBLK):
                js = blk * NB
                nc.gpsimd.partition_broadcast(rstd_b[:, js:js + NB],
                                              rrow[0:1, js:js + NB])
                nc.vector.tensor_tensor(out=xt[:, 0, 1 + js:1 + js + NB],
                                        in0=xt[:, 0, 1 + js:1 + js + NB],
                                        in1=rstd_b[:, js:js + NB], op=AL.mult)
                nc.vector.tensor_tensor(out=xt[:, 1, 1 + js:1 + js + NB],
                                        in0=xt[:, 1, 1 + js:1 + js + NB],
                                        in1=rstd_b[:, js:js + NB], op=AL.mult)

            # ---- fc1 + conv fused on PE: 3 shifted tap matmuls x2 K-chunks
            #      accumulate conv(u) per block; gelu (deferred biases)
            #      evacuates PSUM directly.
            t = tpool.tile([128, 3, L], BF16)
            for blk in range(NBLK):
                cs = blk * NB
                for m in range(3):
                    pt_ = pf1.tile([128, NB], F32, tag="pt_")
                    for k in range(3):
                        nc.tensor.matmul(pt_,
                                         lhsT=w1a[:, k, m * 128:(m + 1) * 128],
                                         rhs=xt[:, 0, cs + k:cs + k + NB],
                                         start=(k == 0), stop=False)
                        nc.tensor.matmul(pt_,
                                         lhsT=w1b[:, k, m * 128:(m + 1) * 128],
                                         rhs=xt[0:65, 1, cs + k:cs + k + NB],
                                         start=False, stop=(k == 2))
                    nc.scalar.activation(out=t[:, m, cs:cs + NB], in_=pt_,
                                         func=AF.Gelu, bias=gb3[:, m, 0:1],
                                         scale=1.0)
                    if blk == 0:
                        nc.scalar.activation(out=t[:, m, 0:1], in_=pt_[:, 0:1],
                                             func=AF.Gelu, bias=gb3[:, m, 1:2],
                                             scale=1.0)
                    if blk == NBLK - 1:
                        nc.scalar.activation(out=t[:, m, L - 1:L],
                                             in_=pt_[:, NB - 1:NB],
                                             func=AF.Gelu, bias=gb3[:, m, 2:3],
                                             scale=1.0)

            # ---- fc2 channel-major: w2 chunks stationary, t moving; emits
            #      F = t @ W2 (bias + residual live on the host).  The
            #      first 4 blocks' output DMA overlaps the tail compute.
            f_sb = fpool.tile([128, 2, L], BF16)
            for blk in range(NBLK):
                cs = blk * NB
                p0 = pf2.tile([128, NB], F32, tag="p0")
                p1 = pf2.tile([64, NB], F32, tag="p1")
                for d3 in range(3):
                    nc.tensor.matmul(p0, lhsT=w2[:, d3, 0:128],
                                     rhs=t[:, d3, cs:cs + NB],
                                     start=(d3 == 0), stop=(d3 == 2))
                for d3 in range(3):
                    nc.tensor.matmul(p1, lhsT=w2[:, d3, 128:192],
                                     rhs=t[:, d3, cs:cs + NB],
                                     start=(d3 == 0), stop=(d3 == 2))
                nc.scalar.copy(out=f_sb[:, 0, cs:cs + NB], in_=p0)
                nc.scalar.copy(out=f_sb[0:64, 1, cs:cs + NB], in_=p1)
                if blk == 3:
                    nc.sync.dma_start(out=f_d[s, :, :, 0:4 * NB],
                                      in_=f_sb[:, :, 0:4 * NB])
                if blk == 5:
                    nc.sync.dma_start(out=f_d[s, :, :, 4 * NB:6 * NB],
                                      in_=f_sb[:, :, 4 * NB:6 * NB])
            nc.sync.dma_start(out=f_d[s, :, :, 6 * NB:L],
                              in_=f_sb[:, :, 6 * NB:L])

        # stats of sample s+1 run on DVE behind sample s's column scales,
        # inside sample s's PE window
        pre_sample(0)
        for s in range(S):
            main_sample(s)
            if s + 1 < S:
                pre_sample(s + 1)
    nc.finalize()
    return nc


def _get_nc():
    if "nc" not in _CACHE:
        _CACHE["nc"] = _build_nc()
    return _CACHE["nc"]


def _host_params(gamma, beta, fc1_w, fc1_b, dw_w, dw_b, fc2_w, fc2_b):
    bf = ml_dtypes.bfloat16
    w1g = (fc1_w * gamma[:, None]).astype(np.float32)          # [192, 384]
    s1g = w1g.sum(0)                                           # [384]
    b1aug = (beta @ fc1_w + fc1_b).astype(np.float32)          # [384]
    dwtaps = dw_w[:, 0, :].T.astype(np.float32)                # [3, 384]
    w1a = (w1g[0:128][:, None, :] * dwtaps[None, :, :]).astype(bf)
    w1b_base = np.concatenate([w1g[128:192], -s1g[None, :]], 0)  # [65, 384]
    w1b = (w1b_base[:, None, :] * dwtaps[None, :, :]).astype(bf)
    w2 = np.ascontiguousarray(
        fc2_w.reshape(3, 128, C).transpose(1, 0, 2)).astype(bf)  # [128,3,192]
    w0, w1_, w2_ = dwtaps[0], dwtaps[1], dwtaps[2]
    gb_int = dw_b + b1aug * (w0 + w1_ + w2_)
    gb_l = dw_b + b1aug * (w1_ + w2_)        # col 0: tap0 falls on zero pad
    gb_r = dw_b + b1aug * (w0 + w1_)         # col L-1: tap2 falls on zero pad
    gb3 = np.ascontiguousarray(
        np.stack([gb_int, gb_l, gb_r], -1).reshape(3, 128, 3)
        .transpose(1, 0, 2)).astype(np.float32)                # [128, 3, 3]
    return dict(w1a=np.ascontiguousarray(w1a),
                w1b=np.ascontiguousarray(w1b), w2=w2, gb3=gb3)


def _host_xt(x_dev):
    """Channel-major bf16 copy of x: [B, 128, 2, L+2] with zero guard columns
    at 0 and L+1 (the conv's zero padding); [:, 64:, 1, :] holds the runtime
    mu row (slot 64) and padding, zero-filled here."""
    bf = ml_dtypes.bfloat16
    nb = x_dev.shape[0]
    arr = np.ascontiguousarray(
        x_dev.reshape(nb, L, C).transpose(0, 2, 1)).astype(bf)  # [nb, 192, L]
    xt = np.zeros((nb, 128, 2, L + 2), dtype=bf)
    xt[:, :, 0, 1:1 + L] = arr[:, 0:128]
    xt[:, 0:64, 1, 1:1 + L] = arr[:, 128:192]
    return xt


def _selector_flags(x, gamma, beta, sel_w1, sel_b1, sel_w2, sel_b2):
    """Exact numpy replica of the reference direction selector. Only used
    when gamma is non-uniform (otherwise the scores tie and idx==0 always)."""
    xf = x.astype(np.float32)
    mu = xf.mean(-1, keepdims=True)
    var = ((xf - mu) ** 2).mean(-1, keepdims=True)
    xn = (xf - mu) / np.sqrt(var + EPS) * gamma + beta
    xg = xn.mean(-1)
    gh = np.abs(xg[:, :, 1:] - xg[:, :, :-1]).mean(axis=(1, 2))
    gv = np.abs(xg[:, 1:, :] - xg[:, :-1, :]).mean(axis=(1, 2))
    scores = np.stack([gh, gv, 0.8 * (gh + gv) * 0.5, np.abs(gh - gv)], 1)
    hdn = np.maximum(scores @ sel_w1 + sel_b1, 0.0)
    logits = hdn @ sel_w2 + sel_b2
    ex = np.exp(logits - logits.max(1, keepdims=True))
    probs = ex / ex.sum(1, keepdims=True)
    return probs.argmax(1) % 4 == 1


def build_in_maps(inputs):
    """Shared by kernel() and test harnesses: host preprocessing + sharding.
    Returns (in_maps, x, x_dev, flags)."""
    bf = ml_dtypes.bfloat16
    x = np.asarray(inputs["x"], dtype=np.float32)
    gamma = np.asarray(inputs["gamma"], np.float32)
    beta = np.asarray(inputs["beta"], np.float32)
    params = _host_params(
        gamma, beta,
        np.asarray(inputs["fc1_w"], np.float32),
        np.asarray(inputs["fc1_b"], np.float32),
        np.asarray(inputs["dw_w"], np.float32),
        np.asarray(inputs["dw_b"], np.float32),
        np.asarray(inputs["fc2_w"], np.float32),
        np.asarray(inputs["fc2_b"], np.float32),
    )

    # Routing: uniform gamma => gray image is constant => scores tie => idx 0
    # for every sample (see module docstring).  Otherwise compute the selector
    # on host and pre-transpose flagged samples (mathematically exact fixup).
    if np.ptp(gamma) == 0.0:
        flags = np.zeros(B, dtype=bool)
    else:
        flags = _selector_flags(
            x, gamma, beta,
            np.asarray(inputs["sel_w1"], np.float32),
            np.asarray(inputs["sel_b1"], np.float32),
            np.asarray(inputs["sel_w2"], np.float32),
            np.asarray(inputs["sel_b2"], np.float32))
    x_dev = x
    if flags.any():
        x_dev = x.copy()
        x_dev[flags] = np.swapaxes(x_dev[flags], 1, 2)

    xt = _host_xt(x_dev)
    xs = x_dev.reshape(B, L, C).astype(bf)
    in_maps = []
    for i in range(NCORES):
        m = {"xs": np.ascontiguousarray(
                 xs[S * i:S * (i + 1)].reshape(S * L, C)),
             "xt": xt[S * i:S * (i + 1)]}
        m.update(params)
        in_maps.append(m)
    return in_maps, x, x_dev, flags


def kernel(**inputs):
    from concourse.bass_utils import run_bass_kernel_spmd

    in_maps, x, x_dev, flags = build_in_maps(inputs)
    nc = _get_nc()
    res = run_bass_kernel_spmd(nc, in_maps, list(range(NCORES)))
    fcm = np.empty((B, 192, L), np.float32)
    for i, r in enumerate(res.results):
        fcm[S * i:S * (i + 1), 0:128] = r["F"][:, :, 0, :]
        fcm[S * i:S * (i + 1), 128:192] = r["F"][:, 0:64, 1, :]
    out = fcm.transpose(0, 2, 1).reshape(B, H, W, C)
    fc2_b = np.asarray(inputs["fc2_b"], np.float32)
    # device computed F(x_dev); reference wants x + F(x_dev) + fc2_b
    # (row-major unscan orientation is identical for flagged samples)
    y = x + out + fc2_b
    return y.astype(np.float32)


# revision 22
# speedup vs baseline: 1.6014x; 1.0218x over previous
"""CASS block (LayerNorm + gradient-selected scan + fc1/dwconv/gelu/fc2 + residual)
on 8 TRN2 NeuronCores, pure data parallel over the batch.

v3 — PE-centric restructure.  Key algebra: with per-pixel LN stats (mu, rstd),
    u = LN(x) @ W1 + b1 = (rstd .* [x; mu]) @ [W1g; -s1g] + b1aug
so the per-pixel rstd is folded into the matmul *rhs* (column scale of the
channel-major input) instead of the psum output (3x larger).  The 3-tap
depthwise conv that follows fc1 is linear, so it is folded into the fc1
matmul itself: conv(u)[l] = sum_k (W1g .* w_k)^T xr[l+k-1], i.e. three
shifted accumulating matmuls per output block against one zero-guarded
channel-major input (guard columns give the conv's zero padding for free).
Gelu (with the fc1/dwconv biases deferred into its bias operand, plus 2
boundary-column fixups) reads the accumulated PSUM directly.  fc2 runs
channel-major with w2 as the stationary operand, and the kernel emits only
the block output F = fc2(gelu(...)); the residual y = x + F + fc2_b is an
elementwise epilogue done host-side in fp32 (exact x, no back-transpose).

Latency details: the rstd row is broadcast across partitions with a K=1
ones-matmul into PSUM (per 392-col block) that the DVE column-scale reads
directly — no gpsimd.  A burst of throwaway matmuls at kernel start keeps
the PE busy while sample 0's stats chain runs, so the HAM clock-gate is
warm (2.4 GHz) when the real fc1 stream begins.  fc1/fc2 are interleaved
per half-image so the output DMA of half 0 overlaps compute of half 1.

The gradient selector: for uniform gamma the "gray" image mean_c(LN(x)) is a
constant, so grad_h = grad_v = 0, the MLP logits tie, softmax gives exactly
0.25 each in fp32, and argmax -> idx 0 for every sample: the 'v' (transpose)
branch is dead.  The device kernel therefore always scans row-major; a host
fallback handles non-uniform gamma by pre-transposing flagged samples (the
row-major reshape of the result is orientation-identical, so y = x + F(x_dev)
recovers the reference output exactly).
"""

import numpy as np
import ml_dtypes

import concourse.mybir as mybir
import concourse.tile as tile
from concourse import bacc

B, H, W, C = 32, 56, 56, 192
D = 384                      # D_INNER
NCORES = 8
S = B // NCORES              # samples per core
L = H * W                    # 3136 pixels per sample
PT = 128                     # pixels per partition tile (stats)
NT = (L + PT - 1) // PT      # 25 pixel tiles (24 full + 64 tail)
TAIL = L - (NT - 1) * PT     # 64
NB = 448                     # columns per matmul block
NBLK = L // NB               # 7 blocks
NWARM = 98                   # HAM warmup matmuls
G = 2                        # left guard columns in xt (even => aligned scales)
KA = 14                      # stats tiles in half A (covers fc1 blocks 0-3)
CA = KA * PT                 # 1792 columns in half A
EPS = 1e-5
F32 = mybir.dt.float32
BF16 = mybir.dt.bfloat16
AL = mybir.AluOpType
AF = mybir.ActivationFunctionType

_CACHE = {}


def _build_nc():
    nc = bacc.Bacc()
    xs_d = nc.declare_dram_parameter("xs", [S * L, C], BF16, isOutput=False)
    xt_d = nc.declare_dram_parameter("xt", [S, 128, 2, L + 4], BF16,
                                     isOutput=False)
    w1a_d = nc.declare_dram_parameter("w1a", [128, 3, D], BF16, isOutput=False)
    w1b_d = nc.declare_dram_parameter("w1b", [65, 3, D], BF16, isOutput=False)
    w2_d = nc.declare_dram_parameter("w2", [128, 3, C], BF16, isOutput=False)
    gb3_d = nc.declare_dram_parameter("gb3", [128, 3, 3], F32, isOutput=False)
    f_d = nc.declare_dram_parameter("F", [S, 128, 2, L], BF16, isOutput=True)

    with tile.TileContext(nc) as tc, \
         tc.tile_pool(name="const", bufs=1) as const, \
         tc.tile_pool(name="xs", bufs=2) as xspool, \
         tc.tile_pool(name="xt", bufs=2) as xtpool, \
         tc.tile_pool(name="stat", bufs=2) as stat, \
         tc.tile_pool(name="t", bufs=2) as tpool, \
         tc.tile_pool(name="f", bufs=2) as fpool, \
         tc.tile_pool(name="rb", bufs=2) as rbpool, \
         tc.tile_pool(name="pf1", bufs=4, space="PSUM") as pf1, \
         tc.tile_pool(name="pf2", bufs=2, space="PSUM") as pf2:

        # ---- HAM warmup: keep the PE busy from t~5us until sample 0's
        #      stats chain delivers real work, so fc1 starts at 2.4 GHz
        #      (an idle gap here re-throttles the clock for ~20us).  Junk
        #      results rotate through the fc1 psum slot, never read.
        junk = const.tile([128, NB], BF16)
        nc.vector.memset(junk, 0.0)
        for _ in range(NWARM):
            jp = pf1.tile([128, NB], F32, name="pt_", tag="pt_")
            nc.tensor.matmul(jp, lhsT=junk[:, 0:128], rhs=junk,
                             start=True, stop=True)

        w1a = const.tile([128, 3, D], BF16)
        w1b = const.tile([65, 3, D], BF16)
        w2 = const.tile([128, 3, C], BF16)
        gb3 = const.tile([128, 3, 3], F32)
        eps_sb = const.tile([128, 1], F32)
        nc.sync.dma_start(out=w1a, in_=w1a_d[:, :, :])
        nc.sync.dma_start(out=w1b, in_=w1b_d[:, :, :])
        nc.sync.dma_start(out=w2, in_=w2_d[:, :, :])
        nc.sync.dma_start(out=gb3, in_=gb3_d[:, :, :])
        nc.vector.memset(eps_sb, EPS)

        state = {}

        def pre_a(s):
            """Stats half A: tiles 0..KA-1 (covers fc1 blocks 0-3 and their
            halo).  Ends with mu/rstd rows for columns [0, CA) in place."""
            base = s * L
            bns = stat.tile([128, NT, 6], F32)
            pack = stat.tile([128, 2, 64], F32)
            packb = stat.tile([128, 2, 64], BF16)
            nc.vector.memset(pack, 0.0)
            nc.vector.memset(packb, 0.0)
            xs_sb = xspool.tile([128, NT, C], BF16)
            for j in range(KA // 2):
                nc.sync.dma_start(
                    out=xs_sb[:, 2 * j:2 * j + 2, :],
                    in_=xs_d[base + j * 256: base + (j + 1) * 256, :]
                        .rearrange("(two p) c -> p two c", p=128),
                )
            xt = xtpool.tile([128, 2, L + 4], BF16)
            nc.sync.dma_start(out=xt, in_=xt_d[s, :, :, :])
            for k in range(KA):
                nc.vector.bn_stats(out=bns[:, k:k + 1, :],
                                   in_=xs_sb[:, k:k + 1, :])
                nc.vector.bn_aggr(out=pack[:, :, k], in_=bns[:, k:k + 1, :])
            nc.scalar.activation(out=pack[:, 1, 0:KA], in_=pack[:, 1, 0:KA],
                                 func=AF.Sqrt, bias=eps_sb[:, :], scale=1.0)
            nc.vector.reciprocal(out=pack[:, 1, 0:KA], in_=pack[:, 1, 0:KA])
            nc.vector.tensor_copy(out=packb[:, :, 0:KA], in_=pack[:, :, 0:KA])
            packT1 = stat.tile([128, 128], BF16)
            nc.sync.dma_start(out=packT1,
                              in_=packb.rearrange("p a b -> p (a b)"),
                              transpose=True)
            rrow = stat.tile([1, L], BF16)
            nc.sync.dma_start(out=xt[64:65, 1, G:G + CA], in_=packT1[0:KA, :])
            nc.sync.dma_start(out=rrow[0:1, 0:CA], in_=packT1[64:64 + KA, :])
            state[s] = (xt, rrow, bns, pack, packb, xs_sb)

        def pre_b(s):
            """Stats half B: tiles KA..NT-1, filling columns [CA, L)."""
            base = s * L
            xt, rrow, bns, pack, packb, xs_sb = state[s]
            for j in range(KA // 2, 12):
                nc.sync.dma_start(
                    out=xs_sb[:, 2 * j:2 * j + 2, :],
                    in_=xs_d[base + j * 256: base + (j + 1) * 256, :]
                        .rearrange("(two p) c -> p two c", p=128),
                )
            nc.sync.dma_start(
                out=xs_sb[0:TAIL, NT - 1, :],
                in_=xs_d[base + (NT - 1) * PT: base + L, :],
            )
            for k in range(KA, NT - 1):
                nc.vector.bn_stats(out=bns[:, k:k + 1, :],
                                   in_=xs_sb[:, k:k + 1, :])
                nc.vector.bn_aggr(out=pack[:, :, k], in_=bns[:, k:k + 1, :])
            nc.vector.bn_stats(out=bns[0:TAIL, NT - 1:NT, :],
                               in_=xs_sb[0:TAIL, NT - 1:NT, :])
            nc.vector.bn_aggr(out=pack[0:TAIL, :, NT - 1],
                              in_=bns[0:TAIL, NT - 1:NT, :])
            nc.scalar.activation(out=pack[:, 1, KA:NT], in_=pack[:, 1, KA:NT],
                                 func=AF.Sqrt, bias=eps_sb[:, :], scale=1.0)
            nc.vector.reciprocal(out=pack[:, 1, KA:NT], in_=pack[:, 1, KA:NT])
            nc.vector.tensor_copy(out=packb[:, :, KA:NT],
                                  in_=pack[:, :, KA:NT])
            packT2 = stat.tile([128, 128], BF16)
            nc.sync.dma_start(out=packT2,
                              in_=packb.rearrange("p a b -> p (a b)"),
                              transpose=True)
            nc.sync.dma_start(out=xt[64:65, 1, G + CA:G + (NT - 1) * PT],
                              in_=packT2[KA:NT - 1, :])
            nc.sync.dma_start(out=xt[64:65, 1, G + (NT - 1) * PT:G + L],
                              in_=packT2[NT - 1:NT, 0:TAIL])
            nc.sync.dma_start(out=rrow[0:1, CA:(NT - 1) * PT],
                              in_=packT2[64 + KA:64 + NT - 1, :])
            nc.sync.dma_start(out=rrow[0:1, (NT - 1) * PT:L],
                              in_=packT2[64 + NT - 1:64 + NT, 0:TAIL])

        def scales(s, rstd_b, blo, bhi):
            """Fold rstd into the fc1 rhs for blocks [blo, bhi): gpsimd
            broadcast (off every critical engine) + DVE 2x column scale."""
            xt, rrow = state[s][0], state[s][1]
            for blk in range(blo, bhi):
                js = blk * NB
                nc.gpsimd.partition_broadcast(rstd_b[:, js:js + NB],
                                              rrow[0:1, js:js + NB])
                nc.vector.tensor_tensor(out=xt[:, 0, G + js:G + js + NB],
                                        in0=xt[:, 0, G + js:G + js + NB],
                                        in1=rstd_b[:, js:js + NB], op=AL.mult)
                nc.vector.tensor_tensor(out=xt[:, 1, G + js:G + js + NB],
                                        in0=xt[:, 1, G + js:G + js + NB],
                                        in1=rstd_b[:, js:js + NB], op=AL.mult)

        def fc1_blk(s, t, blk):
            """fc1 + depthwise conv fused on PE: 3 shifted tap matmuls x2
            K-chunks accumulate conv(u); gelu (deferred biases) evacuates
            PSUM directly."""
            xt = state[s][0]
            cs = blk * NB
            for m in range(3):
                pt_ = pf1.tile([128, NB], F32, tag="pt_")
                for k in range(3):
                    nc.tensor.matmul(pt_,
                                     lhsT=w1a[:, k, m * 128:(m + 1) * 128],
                                     rhs=xt[:, 0, cs + k + 1:cs + k + 1 + NB],
                                     start=(k == 0), stop=False)
                    nc.tensor.matmul(pt_,
                                     lhsT=w1b[:, k, m * 128:(m + 1) * 128],
                                     rhs=xt[0:65, 1, cs + k + 1:cs + k + 1 + NB],
                                     start=False, stop=(k == 2))
                nc.scalar.activation(out=t[:, m, cs:cs + NB], in_=pt_,
                                     func=AF.Gelu, bias=gb3[:, m, 0:1],
                                     scale=1.0)
                if blk == 0:
                    nc.scalar.activation(out=t[:, m, 0:1], in_=pt_[:, 0:1],
                                         func=AF.Gelu, bias=gb3[:, m, 1:2],
                                         scale=1.0)
                if blk == NBLK - 1:
                    nc.scalar.activation(out=t[:, m, L - 1:L],
                                         in_=pt_[:, NB - 1:NB],
                                         func=AF.Gelu, bias=gb3[:, m, 2:3],
                                         scale=1.0)

        def fc2_blk(s, t, f_sb, blk):
            """fc2 channel-major: w2 chunks stationary, t moving; emits
            F = t @ W2 (bias + residual live on the host)."""
            cs = blk * NB
            p0 = pf2.tile([128, NB], F32, tag="p0")
            p1 = pf2.tile([64, NB], F32, tag="p1")
            for d3 in range(3):
                nc.tensor.matmul(p0, lhsT=w2[:, d3, 0:128],
                                 rhs=t[:, d3, cs:cs + NB],
                                 start=(d3 == 0), stop=(d3 == 2))
            for d3 in range(3):
                nc.tensor.matmul(p1, lhsT=w2[:, d3, 128:192],
                                 rhs=t[:, d3, cs:cs + NB],
                                 start=(d3 == 0), stop=(d3 == 2))
            nc.scalar.copy(out=f_sb[:, 0, cs:cs + NB], in_=p0)
            nc.scalar.copy(out=f_sb[0:64, 1, cs:cs + NB], in_=p1)
            if blk == 3:
                nc.sync.dma_start(out=f_d[s, :, :, 0:4 * NB],
                                  in_=f_sb[:, :, 0:4 * NB])
            if blk == 5:
                nc.sync.dma_start(out=f_d[s, :, :, 4 * NB:6 * NB],
                                  in_=f_sb[:, :, 4 * NB:6 * NB])
            if blk == 6:
                nc.sync.dma_start(out=f_d[s, :, :, 6 * NB:L],
                                  in_=f_sb[:, :, 6 * NB:L])

        # Emission order pins each engine's in-order queue: DVE runs
        # scalesA(s) -> bnB(s) -> scalesB(s) -> bnA(s+1); the PE stream
        # (fc1 with fc2 one block behind) never waits on stats.
        pre_a(0)
        for s in range(S):
            rstd_b = rbpool.tile([128, L], BF16)
            t = tpool.tile([128, 3, L], BF16)
            f_sb = fpool.tile([128, 2, L], BF16)
            scales(s, rstd_b, 0, 4)
            pre_b(s)
            fc1_blk(s, t, 0)
            fc1_blk(s, t, 1)
            fc2_blk(s, t, f_sb, 0)
            fc1_blk(s, t, 2)
            fc2_blk(s, t, f_sb, 1)
            scales(s, rstd_b, 4, NBLK)
            if s + 1 < S:
                pre_a(s + 1)
            for blk in range(3, NBLK):
                fc1_blk(s, t, blk)
                fc2_blk(s, t, f_sb, blk - 1)
            fc2_blk(s, t, f_sb, NBLK - 1)
            state.pop(s)
    nc.finalize()
    return nc


def _get_nc():
    if "nc" not in _CACHE:
        _CACHE["nc"] = _build_nc()
    return _CACHE["nc"]


def _host_params(gamma, beta, fc1_w, fc1_b, dw_w, dw_b, fc2_w, fc2_b):
    bf = ml_dtypes.bfloat16
    w1g = (fc1_w * gamma[:, None]).astype(np.float32)          # [192, 384]
    s1g = w1g.sum(0)                                           # [384]
    b1aug = (beta @ fc1_w + fc1_b).astype(np.float32)          # [384]
    dwtaps = dw_w[:, 0, :].T.astype(np.float32)                # [3, 384]
    w1a = (w1g[0:128][:, None, :] * dwtaps[None, :, :]).astype(bf)
    w1b_base = np.concatenate([w1g[128:192], -s1g[None, :]], 0)  # [65, 384]
    w1b = (w1b_base[:, None, :] * dwtaps[None, :, :]).astype(bf)
    w2 = np.ascontiguousarray(
        fc2_w.reshape(3, 128, C).transpose(1, 0, 2)).astype(bf)  # [128,3,192]
    w0, w1_, w2_ = dwtaps[0], dwtaps[1], dwtaps[2]
    gb_int = dw_b + b1aug * (w0 + w1_ + w2_)
    gb_l = dw_b + b1aug * (w1_ + w2_)        # col 0: tap0 falls on zero pad
    gb_r = dw_b + b1aug * (w0 + w1_)         # col L-1: tap2 falls on zero pad
    gb3 = np.ascontiguousarray(
        np.stack([gb_int, gb_l, gb_r], -1).reshape(3, 128, 3)
        .transpose(1, 0, 2)).astype(np.float32)                # [128, 3, 3]
    return dict(w1a=np.ascontiguousarray(w1a),
                w1b=np.ascontiguousarray(w1b), w2=w2, gb3=gb3)


def _host_xt(x_dev):
    """Channel-major bf16 copy of x: [B, 128, 2, L+4] with zero guard columns
    at 0..G-1 and G+L.. (the conv's zero padding; G=2 keeps the scaled region
    4B-aligned); [:, 64:, 1, :] holds the runtime mu row (slot 64) and
    padding, zero-filled here."""
    bf = ml_dtypes.bfloat16
    nb = x_dev.shape[0]
    arr = np.ascontiguousarray(
        x_dev.reshape(nb, L, C).transpose(0, 2, 1)).astype(bf)  # [nb, 192, L]
    xt = np.zeros((nb, 128, 2, L + 4), dtype=bf)
    xt[:, :, 0, G:G + L] = arr[:, 0:128]
    xt[:, 0:64, 1, G:G + L] = arr[:, 128:192]
    return xt


def _selector_flags(x, gamma, beta, sel_w1, sel_b1, sel_w2, sel_b2):
    """Exact numpy replica of the reference direction selector. Only used
    when gamma is non-uniform (otherwise the scores tie and idx==0 always)."""
    xf = x.astype(np.float32)
    mu = xf.mean(-1, keepdims=True)
    var = ((xf - mu) ** 2).mean(-1, keepdims=True)
    xn = (xf - mu) / np.sqrt(var + EPS) * gamma + beta
    xg = xn.mean(-1)
    gh = np.abs(xg[:, :, 1:] - xg[:, :, :-1]).mean(axis=(1, 2))
    gv = np.abs(xg[:, 1:, :] - xg[:, :-1, :]).mean(axis=(1, 2))
    scores = np.stack([gh, gv, 0.8 * (gh + gv) * 0.5, np.abs(gh - gv)], 1)
    hdn = np.maximum(scores @ sel_w1 + sel_b1, 0.0)
    logits = hdn @ sel_w2 + sel_b2
    ex = np.exp(logits - logits.max(1, keepdims=True))
    probs = ex / ex.sum(1, keepdims=True)
    return probs.argmax(1) % 4 == 1


def build_in_maps(inputs):
    """Shared by kernel() and test harnesses: host preprocessing + sharding.
    Returns (in_maps, x, x_dev, flags)."""
    bf = ml_dtypes.bfloat16
    x = np.asarray(inputs["x"], dtype=np.float32)
    gamma = np.asarray(inputs["gamma"], np.float32)
    beta = np.asarray(inputs["beta"], np.float32)
    params = _host_params(
        gamma, beta,
        np.asarray(inputs["fc1_w"], np.float32),
        np.asarray(inputs["fc1_b"], np.float32),
        np.asarray(inputs["dw_w"], np.float32),
        np.asarray(inputs["dw_b"], np.float32),
        np.asarray(inputs["fc2_w"], np.float32),
        np.asarray(inputs["fc2_b"], np.float32),
    )

    # Routing: uniform gamma => gray image is constant => scores tie => idx 0
    # for every sample (see module docstring).  Otherwise compute the selector
    # on host and pre-transpose flagged samples (mathematically exact fixup).
    if np.ptp(gamma) == 0.0:
        flags = np.zeros(B, dtype=bool)
    else:
        flags = _selector_flags(
            x, gamma, beta,
            np.asarray(inputs["sel_w1"], np.float32),
            np.asarray(inputs["sel_b1"], np.float32),
            np.asarray(inputs["sel_w2"], np.float32),
            np.asarray(inputs["sel_b2"], np.float32))
    x_dev = x
    if flags.any():
        x_dev = x.copy()
        x_dev[flags] = np.swapaxes(x_dev[flags], 1, 2)

    xt = _host_xt(x_dev)
    xs = x_dev.reshape(B, L, C).astype(bf)
    in_maps = []
    for i in range(NCORES):
        m = {"xs": np.ascontiguousarray(
                 xs[S * i:S * (i + 1)].reshape(S * L, C)),
             "xt": xt[S * i:S * (i + 1)]}
        m.update(params)
        in_maps.append(m)
    return in_maps, x, x_dev, flags


def kernel(**inputs):
    from concourse.bass_utils import run_bass_kernel_spmd

    in_maps, x, x_dev, flags = build_in_maps(inputs)
    nc = _get_nc()
    res = run_bass_kernel_spmd(nc, in_maps, list(range(NCORES)))
    fcm = np.empty((B, 192, L), np.float32)
    for i, r in enumerate(res.results):
        fcm[S * i:S * (i + 1), 0:128] = r["F"][:, :, 0, :]
        fcm[S * i:S * (i + 1), 128:192] = r["F"][:, 0:64, 1, :]
    out = fcm.transpose(0, 2, 1).reshape(B, H, W, C)
    fc2_b = np.asarray(inputs["fc2_b"], np.float32)
    # device computed F(x_dev); reference wants x + F(x_dev) + fc2_b
    # (row-major unscan orientation is identical for flagged samples)
    y = x + out + fc2_b
    return y.astype(np.float32)
